# revision 2
# baseline (speedup 1.0000x reference)
"""Low-rank self-attention TRN2 kernel v3 — fp8 DoubleRow everywhere.

Tensor-parallel over heads on 8 cores (heads 2c, 2c+1 on core c). Host merges
U@V into per-head effective QKV weights (same FLOPs as the low-rank form at
rank=hidden/2). All big matmuls run as fp8e4 DoubleRow (2x contraction per
pass):

  Stage AB (per 512-col chunk of x): q/k projections (dh-major, two-tier
    hi/lo fp8 on both x and W: hi*hi + lo*hi + hi*lo accumulated in one psum)
    and v projection (seq-major direct, same 3-tier scheme; no transposes).
    q/k evict as [64, 2, S] half-pair layout for the K=64 DoubleRow scores
    trick; v evicts as v_hi + v_lo residual pair (two-tier PV).
  Stage C attention per (qb of 1024 q-cols, head):
    scores S.T[k128, q1024] = DR(k_pair, q_pair)   (K=128 as 2x64 DoubleRow)
    P = exp(2^-15 * S.T) -> fp8e4                  (ACT, the critical engine)
    O += DR(v_hi pair, P pair) + DR(v_lo pair, P pair)   (psum f32)
    r += DR(ones, P pair)                          (row-sum broadcast to 128p)
    t = O/r (DVE divide); O_hi = e4(t); O_lo = e4(t - O_hi)
  o-proj (interleaved into the next qb's pair loop, shares the scores psum
    ring): out[t128, 2048] += DR(O_hi, wo_hi) + DR(O_lo, wo_hi)
    + DR(O_hi, wo_lo); evict f32->bf16 on gpsimd, DMA out.

Scales (powers of 2, folded so PE psums stay in range and exp arg is exact):
  Wq *= 512/sqrt(128), Wk *= 64, Wv *= 64, oW *= 32; exp scale 2^-15;
  host divides the summed partials by 2^11 (= 64*32) and adds o_b.
"""

import math
import sys

sys.path.insert(0, "/opt/trn_rl_repo")

import numpy as np
import ml_dtypes

HIDDEN = 2048
HEADS = 16
DH = 128
S = 4096
NCORES = 8
HPC = HEADS // NCORES   # 2 heads per core
DPC = HPC * DH          # 256 out dims per core per projection
QB = 1024               # q-block in attention
NCHUNK = S // 512       # 8 chunks of 512 seq cols
BF16 = ml_dtypes.bfloat16
E4 = ml_dtypes.float8_e4m3

SQ = 512.0
SK = 64.0
SV = 64.0
SO = 32.0
OUT_DESCALE = 1.0 / (SV * SO)   # applied on host
EXP_SCALE = 1.0 / (SQ * SK)     # 2^-15, exact

_cache = {}


def build_nc(debug=False):
    import concourse.bacc as bacc
    import concourse.mybir as mybir
    import concourse.tile as tile

    dt = mybir.dt
    AF = mybir.ActivationFunctionType
    ALU = mybir.AluOpType
    PM = mybir.MatmulPerfMode

    nc = bacc.Bacc(None, target_bir_lowering=False, debug=debug)
    xhi_d = nc.dram_tensor("xhi", [128, NCHUNK * 16 * 512], dt.float8e4,
                           kind="ExternalInput")
    xlo_d = nc.dram_tensor("xlo", [128, NCHUNK * 16 * 512], dt.float8e4,
                           kind="ExternalInput")
    # q/k are 2-tier ((x_hi+x_lo) @ W_hi); v is 3-tier (needs W_lo too)
    w_ds = {}
    for p, t_ in (("q", "hi"), ("k", "hi"), ("v", "hi"), ("v", "lo")):
        w_ds[p, t_] = nc.dram_tensor(f"w{p}{t_}", [128, 16 * 256],
                                     dt.float8e4, kind="ExternalInput")
    wo_ds = {t_: nc.dram_tensor(f"wo{t_}", [128, HPC * HIDDEN], dt.float8e4,
                                kind="ExternalInput") for t_ in ("hi", "lo")}
    out_d = nc.dram_tensor("out", [S, HIDDEN], dt.bfloat16, kind="ExternalOutput")

    with tile.TileContext(nc) as tc:
        with tc.tile_pool(name="persist", bufs=1) as pp, \
             tc.tile_pool(name="xp", bufs=3) as xp, \
             tc.tile_pool(name="xlp", bufs=3) as xlp, \
             tc.tile_pool(name="pt", bufs=34) as ptp, \
             tc.tile_pool(name="tsb", bufs=1) as tp, \
             tc.tile_pool(name="outst", bufs=2) as osp, \
             tc.tile_pool(name="big", bufs=2, space="PSUM") as bigp, \
             tc.tile_pool(name="oacc", bufs=1, space="PSUM") as oaccp, \
             tc.tile_pool(name="racc", bufs=1, space="PSUM") as raccp:

            # ---- persistent tiles (weight DMAs are emitted in the
            # prologue, interleaved with the first x chunks) ----
            w_s = {}
            for (p, t_), d in w_ds.items():
                w_s[p, t_] = pp.tile([128, 16, 256], dt.float8e4,
                                     tag=f"w{p}{t_}", name=f"w{p}{t_}s")
            wo_s = {}
            for t_, d in wo_ds.items():
                wo_s[t_] = pp.tile([128, HPC, HIDDEN], dt.float8e4,
                                   tag=f"wo{t_}", name=f"wo{t_}s")
            # q/k: [128, 2, S] with head h packed on partitions h*64..h*64+63
            qk_t = {}
            for p in ("q", "k"):
                qk_t[p] = pp.tile([128, 2, S], dt.float8e4,
                                  tag=f"{p}t", name=f"{p}t")
            v_t = {t_: pp.tile([128, S // 128, DPC], dt.float8e4, tag=f"v{t_}",
                               name=f"v{t_}t") for t_ in ("hi", "lo")}
            oT = {t_: pp.tile([128, HPC, S], dt.float8e4, tag=f"oT{t_}",
                              name=f"oT{t_}t") for t_ in ("hi", "lo")}
            ones_t = pp.tile([128, 2, 128], dt.float8e4, tag="ones", name="ones")
            nc.any.memset(ones_t[:], 1.0)
            # preload the exp table off the critical path
            dum_i = pp.tile([1, 16], dt.float32, tag="dumi", name="dumi")
            dum_o = pp.tile([1, 16], dt.bfloat16, tag="dumo", name="dumo")
            nc.any.memset(dum_i[:], 0.0)
            nc.scalar.activation(dum_o[:], dum_i[:], AF.Exp)

            # ---- Stage AB: QKV projection chunk emitters (all psums go
            # through the big ring so oacc/racc are attention-only) ----
            qk_tiers = (("hi", "hi"), ("lo", "hi"))
            v_tiers = (("hi", "hi"), ("lo", "hi"), ("hi", "lo"))

            def emit_chunk_dma(c):
                x_t = {}
                for t_, d in (("hi", xhi_d), ("lo", xlo_d)):
                    pool = xp if t_ == "hi" else xlp
                    x_t[t_] = pool.tile([128, 16, 512], dt.float8e4,
                                        tag=f"x{t_}", name=f"x{t_}_{c}")
                    nc.sync.dma_start(
                        out=x_t[t_][:],
                        in_=d[:, c * 8192:(c + 1) * 8192],
                    )
                return x_t

            def emit_chunk_kq(c, x_t):
                for p in ("k", "q"):
                    if p == "k":
                        kpool, ktag = ((raccp, "racc") if c % 2 == 0
                                       else (oaccp, "oacc"))
                        ps = kpool.tile([128, 1024], dt.float32, tag=ktag,
                                        name=f"{p}ps_{c}")
                    else:
                        ps = bigp.tile([128, 1024], dt.float32, tag="big",
                                       name=f"{p}ps_{c}")
                    for b in range(HPC):
                        reg = ps[:, b * 512:(b + 1) * 512]
                        for ti, (xt_, wt_) in enumerate(qk_tiers):
                            for j2 in range(8):
                                nc.tensor.matmul(
                                    reg,
                                    w_s[p, wt_][:, 2 * j2:2 * j2 + 2,
                                                b * 128:(b + 1) * 128],
                                    x_t[xt_][:, 2 * j2:2 * j2 + 2, :],
                                    start=(ti == 0 and j2 == 0),
                                    stop=(ti == len(qk_tiers) - 1 and j2 == 7),
                                    perf_mode=PM.DoubleRow,
                                    skip_group_check=True,
                                )
                        # evict halves: psum rows 0:64 -> pair half 0,
                        # rows 64:128 -> pair half 1 (partitions b*64..)
                        bsl = slice(b * 64, (b + 1) * 64)
                        nc.vector.tensor_copy(
                            qk_t[p][bsl, 0, c * 512:(c + 1) * 512], reg[0:64, :])
                        nc.vector.tensor_copy(
                            qk_t[p][bsl, 1, c * 512:(c + 1) * 512], reg[64:128, :])
            def emit_chunk_v(c, x_t):
                # v psums use the oacc/racc rings, which are idle until the
                # first attention PVs (those start only after all chunks)
                vpool, vtag = ((oaccp, "oacc") if c % 2 == 0 else (raccp, "racc"))
                vps = vpool.tile([128, 1024], dt.float32, tag=vtag,
                                 name=f"vps_{c}")
                for s_ in range(4):
                    reg = vps[:, s_ * 256:(s_ + 1) * 256]
                    for ti, (xt_, wt_) in enumerate(v_tiers):
                        for j2 in range(8):
                            nc.tensor.matmul(
                                reg,
                                x_t[xt_][:, 2 * j2:2 * j2 + 2,
                                         s_ * 128:(s_ + 1) * 128],
                                w_s["v", wt_][:, 2 * j2:2 * j2 + 2, :],
                                start=(ti == 0 and j2 == 0),
                                stop=(ti == 2 and j2 == 7),
                                perf_mode=PM.DoubleRow,
                                skip_group_check=True,
                            )
                    kb = c * 4 + s_
                    nc.vector.tensor_copy(v_t["hi"][:, kb, :], reg)
                    nc.vector.tensor_tensor(v_t["lo"][:, kb, :], reg,
                                            v_t["hi"][:, kb, :], ALU.subtract)

            # ---- Stage C: attention + fused o-proj ----
            # o-proj work units for qb are emitted interleaved into qb+1's
            # pair loop (and flushed at the end for the last qb).
            pending_oproj = []
            staging = {}

            def emit_oproj_unit(qb, t_blk, half):
                tg = qb * (QB // 128) + t_blk
                if qb == S // QB - 1:
                    pool, ptag = [(bigp, "big"), (oaccp, "oacc"),
                                  (raccp, "racc")][(2 * t_blk + half) % 3]
                else:
                    pool, ptag = bigp, "big"
                ps = pool.tile([128, 1024], dt.float32, tag=ptag,
                               name=f"ops_{tg}_{half}")
                for nb2 in range(2):
                    nb = half * 2 + nb2
                    reg = ps[:, nb2 * 512:(nb2 + 1) * 512]
                    combos = (("hi", "hi"), ("lo", "hi"), ("hi", "lo"))
                    for ci, (ot_, wt_) in enumerate(combos):
                        nc.tensor.matmul(
                            reg,
                            oT[ot_][:, :, tg * 128:(tg + 1) * 128],
                            wo_s[wt_][:, :, nb * 512:(nb + 1) * 512],
                            start=(ci == 0),
                            stop=(ci == 2),
                            perf_mode=PM.DoubleRow,
                            skip_group_check=True,
                        )
                if half == 0:
                    st = osp.tile([128, 2048], dt.bfloat16, tag="outst",
                                  name=f"ost_{tg}")
                    staging[tg] = st
                else:
                    st = staging.pop(tg)
                nc.vector.tensor_copy(st[:, half * 1024:(half + 1) * 1024],
                                      ps[:])
                if half == 1:
                    nc.sync.dma_start(out=out_d[tg * 128:(tg + 1) * 128, :],
                                      in_=st[:])

            def drain_oproj():
                if pending_oproj:
                    qb, t_blk, half = pending_oproj.pop(0)
                    emit_oproj_unit(qb, t_blk, half)

            # Attention is emitted as sweeps (one (qb, h) = 16 pair units).
            # PV/r of sweep s-1 interleave with scores/exp of sweep s (a
            # full-sweep software pipeline), so the PE's in-order queue never
            # stalls the ACT feed; sweep 0's scores chase the QKV chunk
            # production, overlapping stage AB with attention.
            acc_ps = {}

            def emit_scores(u):
                qb, h, pr = u
                p_t = ptp.tile([128, 2, QB], dt.float8e4, tag="pt",
                               name=f"pt_{qb}_{h}_{pr}")
                for i in range(2):
                    kb = 2 * pr + i
                    sc = bigp.tile([128, QB], dt.float32, tag="big",
                                   name=f"sc_{qb}_{h}_{kb}")
                    hsl = slice(h * 64, (h + 1) * 64)
                    for j in range(2):
                        nc.tensor.matmul(
                            sc[:, j * 512:(j + 1) * 512],
                            qk_t["k"][hsl, :, kb * 128:(kb + 1) * 128],
                            qk_t["q"][hsl, :, qb * QB + j * 512:
                                      qb * QB + (j + 1) * 512],
                            start=True,
                            stop=True,
                            perf_mode=PM.DoubleRow,
                            skip_group_check=True,
                        )
                    nc.scalar.activation(p_t[:, i, :], sc[:], AF.Exp,
                                         scale=EXP_SCALE)
                return p_t

            def emit_pv(u, p_t):
                qb, h, pr = u
                if pr == 0:
                    acc_ps[qb, h] = (
                        oaccp.tile([128, QB], dt.float32, tag="oacc",
                                   name=f"ops_{qb}_{h}"),
                        raccp.tile([128, QB], dt.float32, tag="racc",
                                   name=f"rps_{qb}_{h}"),
                    )
                o_ps, r_ps = acc_ps[qb, h]
                for j in range(2):
                    jsl = slice(j * 512, (j + 1) * 512)
                    for vi, vt_ in enumerate(("hi", "lo")):
                        nc.tensor.matmul(
                            o_ps[:, jsl],
                            v_t[vt_][:, 2 * pr:2 * pr + 2,
                                     h * 128:(h + 1) * 128],
                            p_t[:, :, jsl],
                            start=(pr == 0 and vi == 0),
                            stop=(pr == 15 and vi == 1),
                            perf_mode=PM.DoubleRow,
                            skip_group_check=True,
                        )
                    nc.tensor.matmul(
                        r_ps[:, jsl],
                        ones_t[:],
                        p_t[:, :, jsl],
                        start=(pr == 0),
                        stop=(pr == 15),
                        perf_mode=PM.DoubleRow,
                        skip_group_check=True,
                    )
                if pr == 15:
                    # softmax normalize + two-tier O eviction
                    del acc_ps[qb, h]
                    rb_sb = tp.tile([128, QB], dt.float32, tag="rbin",
                                    name=f"rb_{qb}_{h}")
                    nc.vector.reciprocal(rb_sb[:], r_ps[:])
                    t_sb = tp.tile([128, QB], dt.bfloat16, tag="tsb",
                                   name=f"tsb_{qb}_{h}")
                    nc.vector.tensor_tensor(t_sb[:], o_ps[:], rb_sb[:],
                                            ALU.mult)
                    ohi = oT["hi"][:, h, qb * QB:(qb + 1) * QB]
                    nc.gpsimd.tensor_copy(ohi, t_sb[:])
                    nc.gpsimd.tensor_tensor(
                        oT["lo"][:, h, qb * QB:(qb + 1) * QB],
                        t_sb[:], ohi, ALU.subtract)
                    if h == HPC - 1:
                        for t_blk in range(QB // 128):
                            pending_oproj.append((qb, t_blk, 0))
                            pending_oproj.append((qb, t_blk, 1))

            sweeps = [(qb, h) for qb in range(S // QB) for h in range(HPC)]
            sc_q = []

            def emit_sc_unit(sw, pr):
                qb, h = sw
                sc_q.append(((qb, h, pr), emit_scores((qb, h, pr))))

            def emit_pv_unit():
                u, p_t = sc_q.pop(0)
                emit_pv(u, p_t)

            # Prologue: stage AB chunks with sweeps 0 AND 1 prefetching their
            # scores/exp behind the chunk production (both depend only on the
            # k/v/q chunks produced so far; no PVs run during AB so the
            # oacc/racc rings are free for the v psums).
            x0 = emit_chunk_dma(0)
            for p, t_ in (("k", "hi"), ("q", "hi"), ("v", "hi"), ("v", "lo")):
                nc.sync.dma_start(out=w_s[p, t_][:], in_=w_ds[p, t_][:])
            x1 = emit_chunk_dma(1)
            for t_ in ("hi", "lo"):
                nc.sync.dma_start(out=wo_s[t_][:], in_=wo_ds[t_][:])
            x_tiles = {0: x0, 1: x1}
            emit_chunk_kq(0, x0)
            emit_chunk_kq(1, x1)
            emit_sc_unit(sweeps[0], 0)
            emit_sc_unit(sweeps[0], 1)
            for c in range(2, NCHUNK):
                x_tiles[c] = emit_chunk_dma(c)
                emit_chunk_v(c - 2, x_tiles.pop(c - 2))
                emit_chunk_kq(c, x_tiles[c])
                emit_sc_unit(sweeps[0], 2 * (c - 1))
                emit_sc_unit(sweeps[0], 2 * (c - 1) + 1)
                if c >= 3:
                    emit_sc_unit(sweeps[1], 2 * (c - 3))
                    emit_sc_unit(sweeps[1], 2 * (c - 3) + 1)
                if c >= 4:
                    emit_sc_unit(sweeps[2], 2 * (c - 4))
                    emit_sc_unit(sweeps[2], 2 * (c - 4) + 1)
            emit_chunk_v(NCHUNK - 2, x_tiles.pop(NCHUNK - 2))
            emit_chunk_v(NCHUNK - 1, x_tiles.pop(NCHUNK - 1))
            # post-AB: finish the prefetched sweeps' scores while draining
            # queued PVs (two per unit) to bound the pipeline depth
            tail_units = ([(sweeps[0], pr) for pr in (14, 15)]
                          + [(sweeps[1], pr)
                             for pr in range(2 * (NCHUNK - 3), 16)]
                          + [(sweeps[2], pr)
                             for pr in range(2 * (NCHUNK - 4), 16)])
            for sw, pr in tail_units:
                emit_sc_unit(sw, pr)
                emit_pv_unit()
                if len(sc_q) > 24:
                    emit_pv_unit()
            # Steady state: emit scores of sweep si while draining queued PVs
            # (at most 2 per step) until the pipeline lag settles at one sweep.
            for si in range(3, len(sweeps)):
                tgt = 17 if si < len(sweeps) - 2 else (10 if si == 6 else 4)
                for pr in range(16):
                    if pr == 0:
                        emit_sc_unit(sweeps[si], 0)
                        emit_sc_unit(sweeps[si], 1)
                    elif pr < 15:
                        emit_sc_unit(sweeps[si], pr + 1)
                    emit_pv_unit()
                    if len(sc_q) > tgt:
                        emit_pv_unit()
                    if pr % 2 == 1:
                        drain_oproj()
            while sc_q:
                emit_pv_unit()
                drain_oproj()
            while pending_oproj:
                drain_oproj()
    nc.finalize()
    return nc


def host_prep(hidden_states, q_V, q_U, k_V, k_U, v_V, v_U, o_W):
    """Per-core input maps: fp8 hi/lo splits + swizzled layouts."""
    x = np.asarray(hidden_states, np.float32).reshape(S, HIDDEN)
    xT = np.ascontiguousarray(x.T)                      # [HIDDEN, S]
    x_hi = np.clip(xT, -240, 240).astype(E4)
    x_lo = np.clip(xT - x_hi.astype(np.float32), -240, 240).astype(E4)

    def x_image(a):  # [2048, 4096] -> [128, NCHUNK*16*512] chunk-major swizzle
        # xs[p, c, blk, col] = a[blk*128+p, c*512+col]
        b = a.reshape(16, 128, NCHUNK, 512)             # [blk, p, c, col]
        return np.ascontiguousarray(
            b.transpose(1, 2, 0, 3).reshape(128, NCHUNK * 16 * 512))

    Wq = (np.asarray(q_U, np.float32) @ np.asarray(q_V, np.float32)) \
        * (SQ / math.sqrt(DH))
    Wk = (np.asarray(k_U, np.float32) @ np.asarray(k_V, np.float32)) * SK
    Wv = (np.asarray(v_U, np.float32) @ np.asarray(v_V, np.float32)) * SV
    oW = np.asarray(o_W, np.float32) * SO

    def hilo(a):
        hi = np.clip(a, -240, 240).astype(E4)
        lo = np.clip(a - hi.astype(np.float32), -240, 240).astype(E4)
        return hi, lo

    def w_image(WT):  # [2048, 256] -> [128, 16*256]
        return np.ascontiguousarray(
            WT.reshape(16, 128, DPC).transpose(1, 0, 2).reshape(128, 16 * DPC))

    def wo_image(oWcT):  # [256, 2048] -> [128, 2*2048]
        return np.ascontiguousarray(
            oWcT.reshape(HPC, 128, HIDDEN).transpose(1, 0, 2)
            .reshape(128, HPC * HIDDEN))

    xhi_img = x_image(x_hi)
    xlo_img = x_image(x_lo)
    in_maps = []
    for c in range(NCORES):
        sl = slice(c * DPC, (c + 1) * DPC)
        m = {"xhi": xhi_img, "xlo": xlo_img}
        for p, W in (("q", Wq), ("k", Wk), ("v", Wv)):
            hi, lo = hilo(np.ascontiguousarray(W[sl, :].T))
            m[f"w{p}hi"] = w_image(hi)
            if p == "v":
                m[f"w{p}lo"] = w_image(lo)
        hi, lo = hilo(np.ascontiguousarray(oW[:, sl].T))
        m["wohi"] = wo_image(hi)
        m["wolo"] = wo_image(lo)
        in_maps.append(m)
    return in_maps


def run(inputs, trace=False, tmpdir=None):
    from concourse.bass_utils import run_bass_kernel_spmd

    if "nc" not in _cache:
        _cache["nc"] = build_nc()
    nc = _cache["nc"]
    in_maps = host_prep(
        inputs["hidden_states"], inputs["q_V"], inputs["q_U"], inputs["k_V"],
        inputs["k_U"], inputs["v_V"], inputs["v_U"], inputs["o_W"],
    )
    res = run_bass_kernel_spmd(
        nc, in_maps, core_ids=list(range(NCORES)), trace=trace, tmpdir=tmpdir
    )
    acc = np.zeros((S, HIDDEN), np.float64)
    for c in range(NCORES):
        acc += res.results[c]["out"].astype(np.float64)
    out = (acc * OUT_DESCALE
           + np.asarray(inputs["o_b"], np.float64)[None, :]).astype(np.float32)
    return out.reshape(1, S, HIDDEN), res


def kernel(**inputs) -> np.ndarray:
    out, _ = run(inputs, trace=False)
    return out


# revision 3
# speedup vs baseline: 1.0314x; 1.0314x over previous
"""Low-rank self-attention TRN2 kernel v3 — fp8 DoubleRow everywhere.

Tensor-parallel over heads on 8 cores (heads 2c, 2c+1 on core c). Host merges
U@V into per-head effective QKV weights (same FLOPs as the low-rank form at
rank=hidden/2). All big matmuls run as fp8e4 DoubleRow (2x contraction per
pass):

  Stage AB (per 512-col chunk of x): q/k projections (dh-major, two-tier
    hi/lo fp8 on both x and W: hi*hi + lo*hi + hi*lo accumulated in one psum)
    and v projection (seq-major direct, same 3-tier scheme; no transposes).
    q/k evict as [64, 2, S] half-pair layout for the K=64 DoubleRow scores
    trick; v evicts as v_hi + v_lo residual pair (two-tier PV).
  Stage C attention per (qb of 1024 q-cols, head):
    scores S.T[k128, q1024] = DR(k_pair, q_pair)   (K=128 as 2x64 DoubleRow)
    P = exp(2^-15 * S.T) -> fp8e4                  (ACT, the critical engine)
    O += DR(v_hi pair, P pair) + DR(v_lo pair, P pair)   (psum f32)
    r += DR(ones, P pair)                          (row-sum broadcast to 128p)
    t = O/r (DVE divide); O_hi = e4(t); O_lo = e4(t - O_hi)
  o-proj (interleaved into the next qb's pair loop, shares the scores psum
    ring): out[t128, 2048] += DR(O_hi, wo_hi) + DR(O_lo, wo_hi)
    + DR(O_hi, wo_lo); evict f32->bf16 on gpsimd, DMA out.

Scales (powers of 2, folded so PE psums stay in range and exp arg is exact):
  Wq *= 512/sqrt(128), Wk *= 64, Wv *= 64, oW *= 32; exp scale 2^-15;
  host divides the summed partials by 2^11 (= 64*32) and adds o_b.
"""

import math
import sys

sys.path.insert(0, "/opt/trn_rl_repo")

import numpy as np
import ml_dtypes

HIDDEN = 2048
HEADS = 16
DH = 128
S = 4096
NCORES = 8
HPC = HEADS // NCORES   # 2 heads per core
DPC = HPC * DH          # 256 out dims per core per projection
QB = 1024               # q-block in attention
NCHUNK = S // 512       # 8 chunks of 512 seq cols
BF16 = ml_dtypes.bfloat16
E4 = ml_dtypes.float8_e4m3

SQ = 512.0
SK = 64.0
SV = 64.0
SO = 32.0
OUT_DESCALE = 1.0 / (SV * SO)   # applied on host
EXP_SCALE = 1.0 / (SQ * SK)     # 2^-15, exact

_cache = {}


def build_nc(debug=False):
    import concourse.bacc as bacc
    import concourse.mybir as mybir
    import concourse.tile as tile

    dt = mybir.dt
    AF = mybir.ActivationFunctionType
    ALU = mybir.AluOpType
    PM = mybir.MatmulPerfMode

    nc = bacc.Bacc(None, target_bir_lowering=False, debug=debug)
    xhi_d = nc.dram_tensor("xhi", [128, NCHUNK * 16 * 512], dt.float8e4,
                           kind="ExternalInput")
    xlo_d = nc.dram_tensor("xlo", [128, NCHUNK * 16 * 512], dt.float8e4,
                           kind="ExternalInput")
    # q/k are 2-tier ((x_hi+x_lo) @ W_hi); v is 3-tier (needs W_lo too)
    w_ds = {}
    for p, t_ in (("q", "hi"), ("k", "hi"), ("v", "hi"), ("v", "lo")):
        w_ds[p, t_] = nc.dram_tensor(f"w{p}{t_}", [128, 16 * 256],
                                     dt.float8e4, kind="ExternalInput")
    wo_ds = {t_: nc.dram_tensor(f"wo{t_}", [128, HPC * HIDDEN], dt.float8e4,
                                kind="ExternalInput") for t_ in ("hi", "lo")}
    out_d = nc.dram_tensor("out", [S, HIDDEN], dt.bfloat16, kind="ExternalOutput")

    with tile.TileContext(nc) as tc:
        with tc.tile_pool(name="persist", bufs=1) as pp, \
             tc.tile_pool(name="xp", bufs=3) as xp, \
             tc.tile_pool(name="xlp", bufs=3) as xlp, \
             tc.tile_pool(name="pt", bufs=34) as ptp, \
             tc.tile_pool(name="tsb", bufs=1) as tp, \
             tc.tile_pool(name="outst", bufs=3) as osp, \
             tc.tile_pool(name="big", bufs=2, space="PSUM") as bigp, \
             tc.tile_pool(name="oacc", bufs=1, space="PSUM") as oaccp, \
             tc.tile_pool(name="racc", bufs=1, space="PSUM") as raccp:

            # ---- persistent tiles (weight DMAs are emitted in the
            # prologue, interleaved with the first x chunks) ----
            w_s = {}
            for (p, t_), d in w_ds.items():
                w_s[p, t_] = pp.tile([128, 16, 256], dt.float8e4,
                                     tag=f"w{p}{t_}", name=f"w{p}{t_}s")
            wo_s = {}
            for t_, d in wo_ds.items():
                wo_s[t_] = pp.tile([128, HPC, HIDDEN], dt.float8e4,
                                   tag=f"wo{t_}", name=f"wo{t_}s")
            # q/k: [128, 2, S] with head h packed on partitions h*64..h*64+63
            qk_t = {}
            for p in ("q", "k"):
                qk_t[p] = pp.tile([128, 2, S], dt.float8e4,
                                  tag=f"{p}t", name=f"{p}t")
            v_t = {t_: pp.tile([128, S // 128, DPC], dt.float8e4, tag=f"v{t_}",
                               name=f"v{t_}t") for t_ in ("hi", "lo")}
            oT = {t_: pp.tile([128, HPC, S], dt.float8e4, tag=f"oT{t_}",
                              name=f"oT{t_}t") for t_ in ("hi", "lo")}
            ones_t = pp.tile([128, 2, 128], dt.float8e4, tag="ones", name="ones")
            nc.any.memset(ones_t[:], 1.0)
            # preload the exp table off the critical path
            dum_i = pp.tile([1, 16], dt.float32, tag="dumi", name="dumi")
            dum_o = pp.tile([1, 16], dt.bfloat16, tag="dumo", name="dumo")
            nc.any.memset(dum_i[:], 0.0)
            nc.scalar.activation(dum_o[:], dum_i[:], AF.Exp)

            # ---- Stage AB: QKV projection chunk emitters (all psums go
            # through the big ring so oacc/racc are attention-only) ----
            qk_tiers = (("hi", "hi"), ("lo", "hi"))
            v_tiers = (("hi", "hi"), ("lo", "hi"), ("hi", "lo"))

            def emit_chunk_dma(c, split=1):
                x_t = {}
                for t_, d in (("hi", xhi_d), ("lo", xlo_d)):
                    pool = xp if t_ == "hi" else xlp
                    x_t[t_] = pool.tile([128, 16, 512], dt.float8e4,
                                        tag=f"x{t_}", name=f"x{t_}_{c}")
                    sub = 8192 // split
                    for qtr in range(split):
                        nc.sync.dma_start(
                            out=x_t[t_][:, qtr * (16 // split):
                                        (qtr + 1) * (16 // split), :],
                            in_=d[:, c * 8192 + qtr * sub:
                                  c * 8192 + (qtr + 1) * sub],
                        )
                return x_t

            def emit_chunk_kq(c, x_t):
                for p in ("k", "q"):
                    if p == "k":
                        kpool, ktag = ((raccp, "racc") if c % 2 == 0
                                       else (oaccp, "oacc"))
                        ps = kpool.tile([128, 1024], dt.float32, tag=ktag,
                                        name=f"{p}ps_{c}")
                    else:
                        ps = bigp.tile([128, 1024], dt.float32, tag="big",
                                       name=f"{p}ps_{c}")
                    for b in range(HPC):
                        reg = ps[:, b * 512:(b + 1) * 512]
                        for ti, (xt_, wt_) in enumerate(qk_tiers):
                            for j2 in range(8):
                                nc.tensor.matmul(
                                    reg,
                                    w_s[p, wt_][:, 2 * j2:2 * j2 + 2,
                                                b * 128:(b + 1) * 128],
                                    x_t[xt_][:, 2 * j2:2 * j2 + 2, :],
                                    start=(ti == 0 and j2 == 0),
                                    stop=(ti == len(qk_tiers) - 1 and j2 == 7),
                                    perf_mode=PM.DoubleRow,
                                    skip_group_check=True,
                                )
                        # evict halves: psum rows 0:64 -> pair half 0,
                        # rows 64:128 -> pair half 1 (partitions b*64..)
                        bsl = slice(b * 64, (b + 1) * 64)
                        nc.vector.tensor_copy(
                            qk_t[p][bsl, 0, c * 512:(c + 1) * 512], reg[0:64, :])
                        nc.vector.tensor_copy(
                            qk_t[p][bsl, 1, c * 512:(c + 1) * 512], reg[64:128, :])
            def emit_chunk_v(c, x_t):
                # v psums use the oacc/racc rings, which are idle until the
                # first attention PVs (those start only after all chunks)
                vpool, vtag = ((oaccp, "oacc") if c % 2 == 0 else (raccp, "racc"))
                vps = vpool.tile([128, 1024], dt.float32, tag=vtag,
                                 name=f"vps_{c}")
                for s_ in range(4):
                    reg = vps[:, s_ * 256:(s_ + 1) * 256]
                    for ti, (xt_, wt_) in enumerate(v_tiers):
                        for j2 in range(8):
                            nc.tensor.matmul(
                                reg,
                                x_t[xt_][:, 2 * j2:2 * j2 + 2,
                                         s_ * 128:(s_ + 1) * 128],
                                w_s["v", wt_][:, 2 * j2:2 * j2 + 2, :],
                                start=(ti == 0 and j2 == 0),
                                stop=(ti == 2 and j2 == 7),
                                perf_mode=PM.DoubleRow,
                                skip_group_check=True,
                            )
                    kb = c * 4 + s_
                    nc.vector.tensor_copy(v_t["hi"][:, kb, :], reg)
                    nc.vector.tensor_tensor(v_t["lo"][:, kb, :], reg,
                                            v_t["hi"][:, kb, :], ALU.subtract)

            # ---- Stage C: attention + fused o-proj ----
            # o-proj work units for qb are emitted interleaved into qb+1's
            # pair loop (and flushed at the end for the last qb).
            pending_oproj = []
            staging = {}

            def emit_oproj_unit(qb, t_blk, half):
                tg = qb * (QB // 128) + t_blk
                if qb == S // QB - 1:
                    pool, ptag = [(bigp, "big"), (oaccp, "oacc"),
                                  (raccp, "racc")][(2 * t_blk + half) % 3]
                else:
                    pool, ptag = bigp, "big"
                ps = pool.tile([128, 1024], dt.float32, tag=ptag,
                               name=f"ops_{tg}_{half}")
                for nb2 in range(2):
                    nb = half * 2 + nb2
                    reg = ps[:, nb2 * 512:(nb2 + 1) * 512]
                    combos = (("hi", "hi"), ("lo", "hi"), ("hi", "lo"))
                    for ci, (ot_, wt_) in enumerate(combos):
                        nc.tensor.matmul(
                            reg,
                            oT[ot_][:, :, tg * 128:(tg + 1) * 128],
                            wo_s[wt_][:, :, nb * 512:(nb + 1) * 512],
                            start=(ci == 0),
                            stop=(ci == 2),
                            perf_mode=PM.DoubleRow,
                            skip_group_check=True,
                        )
                if half == 0:
                    st = osp.tile([128, 2048], dt.bfloat16, tag="outst",
                                  name=f"ost_{tg}")
                    staging[tg] = st
                else:
                    st = staging.pop(tg)
                if qb == S // QB - 1 and (2 * t_blk + half) % 2 == 0:
                    nc.scalar.copy(st[:, half * 1024:(half + 1) * 1024], ps[:])
                else:
                    nc.vector.tensor_copy(
                        st[:, half * 1024:(half + 1) * 1024], ps[:])
                if half == 1:
                    nc.sync.dma_start(out=out_d[tg * 128:(tg + 1) * 128, :],
                                      in_=st[:])

            def drain_oproj():
                if pending_oproj:
                    qb, t_blk, half = pending_oproj.pop(0)
                    emit_oproj_unit(qb, t_blk, half)

            # Attention is emitted as sweeps (one (qb, h) = 16 pair units).
            # PV/r of sweep s-1 interleave with scores/exp of sweep s (a
            # full-sweep software pipeline), so the PE's in-order queue never
            # stalls the ACT feed; sweep 0's scores chase the QKV chunk
            # production, overlapping stage AB with attention.
            acc_ps = {}

            def emit_scores(u):
                qb, h, pr = u
                p_t = ptp.tile([128, 2, QB], dt.float8e4, tag="pt",
                               name=f"pt_{qb}_{h}_{pr}")
                for i in range(2):
                    kb = 2 * pr + i
                    sc = bigp.tile([128, QB], dt.float32, tag="big",
                                   name=f"sc_{qb}_{h}_{kb}")
                    hsl = slice(h * 64, (h + 1) * 64)
                    for j in range(2):
                        nc.tensor.matmul(
                            sc[:, j * 512:(j + 1) * 512],
                            qk_t["k"][hsl, :, kb * 128:(kb + 1) * 128],
                            qk_t["q"][hsl, :, qb * QB + j * 512:
                                      qb * QB + (j + 1) * 512],
                            start=True,
                            stop=True,
                            perf_mode=PM.DoubleRow,
                            skip_group_check=True,
                        )
                    nc.scalar.activation(p_t[:, i, :], sc[:], AF.Exp,
                                         scale=EXP_SCALE)
                return p_t

            def emit_pv(u, p_t):
                qb, h, pr = u
                if pr == 0:
                    acc_ps[qb, h] = (
                        oaccp.tile([128, QB], dt.float32, tag="oacc",
                                   name=f"ops_{qb}_{h}"),
                        raccp.tile([128, QB], dt.float32, tag="racc",
                                   name=f"rps_{qb}_{h}"),
                    )
                o_ps, r_ps = acc_ps[qb, h]
                for j in range(2):
                    jsl = slice(j * 512, (j + 1) * 512)
                    for vi, vt_ in enumerate(("hi", "lo")):
                        nc.tensor.matmul(
                            o_ps[:, jsl],
                            v_t[vt_][:, 2 * pr:2 * pr + 2,
                                     h * 128:(h + 1) * 128],
                            p_t[:, :, jsl],
                            start=(pr == 0 and vi == 0),
                            stop=(pr == 15 and vi == 1),
                            perf_mode=PM.DoubleRow,
                            skip_group_check=True,
                        )
                    nc.tensor.matmul(
                        r_ps[:, jsl],
                        ones_t[:],
                        p_t[:, :, jsl],
                        start=(pr == 0),
                        stop=(pr == 15),
                        perf_mode=PM.DoubleRow,
                        skip_group_check=True,
                    )
                if pr == 15:
                    # softmax normalize + two-tier O eviction
                    del acc_ps[qb, h]
                    rb_sb = tp.tile([128, QB], dt.float32, tag="rbin",
                                    name=f"rb_{qb}_{h}")
                    nc.vector.reciprocal(rb_sb[:], r_ps[:])
                    t_sb = tp.tile([128, QB], dt.bfloat16, tag="tsb",
                                   name=f"tsb_{qb}_{h}")
                    nc.vector.tensor_tensor(t_sb[:], o_ps[:], rb_sb[:],
                                            ALU.mult)
                    ohi = oT["hi"][:, h, qb * QB:(qb + 1) * QB]
                    nc.gpsimd.tensor_copy(ohi, t_sb[:])
                    nc.gpsimd.tensor_tensor(
                        oT["lo"][:, h, qb * QB:(qb + 1) * QB],
                        t_sb[:], ohi, ALU.subtract)
                    if h == HPC - 1:
                        for t_blk in range(QB // 128):
                            pending_oproj.append((qb, t_blk, 0))
                            pending_oproj.append((qb, t_blk, 1))

            sweeps = [(qb, h) for qb in range(S // QB) for h in range(HPC)]
            sc_q = []

            def emit_sc_unit(sw, pr):
                qb, h = sw
                sc_q.append(((qb, h, pr), emit_scores((qb, h, pr))))

            def emit_pv_unit():
                u, p_t = sc_q.pop(0)
                emit_pv(u, p_t)

            # Prologue: stage AB chunks with sweeps 0 AND 1 prefetching their
            # scores/exp behind the chunk production (both depend only on the
            # k/v/q chunks produced so far; no PVs run during AB so the
            # oacc/racc rings are free for the v psums).
            x0 = emit_chunk_dma(0)
            for p, t_ in (("k", "hi"), ("q", "hi"), ("v", "hi"), ("v", "lo")):
                nc.scalar.dma_start(out=w_s[p, t_][:], in_=w_ds[p, t_][:])
            x1 = emit_chunk_dma(1)
            for t_ in ("hi", "lo"):
                nc.scalar.dma_start(out=wo_s[t_][:], in_=wo_ds[t_][:])
            x_tiles = {0: x0, 1: x1}
            emit_chunk_kq(0, x0)
            emit_chunk_kq(1, x1)
            emit_sc_unit(sweeps[0], 0)
            emit_sc_unit(sweeps[0], 1)
            for c in range(2, NCHUNK):
                x_tiles[c] = emit_chunk_dma(c)
                emit_chunk_v(c - 2, x_tiles.pop(c - 2))
                emit_chunk_kq(c, x_tiles[c])
                emit_sc_unit(sweeps[0], 2 * (c - 1))
                emit_sc_unit(sweeps[0], 2 * (c - 1) + 1)
                if c >= 3:
                    emit_sc_unit(sweeps[1], 2 * (c - 3))
                    emit_sc_unit(sweeps[1], 2 * (c - 3) + 1)
                if c >= 4:
                    emit_sc_unit(sweeps[2], 2 * (c - 4))
                    emit_sc_unit(sweeps[2], 2 * (c - 4) + 1)
            emit_chunk_v(NCHUNK - 2, x_tiles.pop(NCHUNK - 2))
            emit_chunk_v(NCHUNK - 1, x_tiles.pop(NCHUNK - 1))
            # post-AB: finish the prefetched sweeps' scores while draining
            # queued PVs (two per unit) to bound the pipeline depth
            tail_units = ([(sweeps[0], pr) for pr in (14, 15)]
                          + [(sweeps[1], pr)
                             for pr in range(2 * (NCHUNK - 3), 16)]
                          + [(sweeps[2], pr)
                             for pr in range(2 * (NCHUNK - 4), 16)])
            for sw, pr in tail_units:
                emit_sc_unit(sw, pr)
                emit_pv_unit()
                if len(sc_q) > 24:
                    emit_pv_unit()
            # Steady state: emit scores of sweep si while draining queued PVs
            # (at most 2 per step) until the pipeline lag settles at one sweep.
            for si in range(3, len(sweeps)):
                tgt = 17 if si < len(sweeps) - 2 else (10 if si == 6 else 4)
                for pr in range(16):
                    if pr == 0:
                        emit_sc_unit(sweeps[si], 0)
                        emit_sc_unit(sweeps[si], 1)
                    elif pr < 15:
                        emit_sc_unit(sweeps[si], pr + 1)
                    emit_pv_unit()
                    if len(sc_q) > tgt:
                        emit_pv_unit()
                    if pr % 2 == 1:
                        drain_oproj()
            while sc_q:
                emit_pv_unit()
                drain_oproj()
            while pending_oproj:
                drain_oproj()
    nc.finalize()
    return nc


def host_prep(hidden_states, q_V, q_U, k_V, k_U, v_V, v_U, o_W):
    """Per-core input maps: fp8 hi/lo splits + swizzled layouts."""
    x = np.asarray(hidden_states, np.float32).reshape(S, HIDDEN)
    xT = np.ascontiguousarray(x.T)                      # [HIDDEN, S]
    x_hi = np.clip(xT, -240, 240).astype(E4)
    x_lo = np.clip(xT - x_hi.astype(np.float32), -240, 240).astype(E4)

    def x_image(a):  # [2048, 4096] -> [128, NCHUNK*16*512] chunk-major swizzle
        # xs[p, c, blk, col] = a[blk*128+p, c*512+col]
        b = a.reshape(16, 128, NCHUNK, 512)             # [blk, p, c, col]
        return np.ascontiguousarray(
            b.transpose(1, 2, 0, 3).reshape(128, NCHUNK * 16 * 512))

    Wq = (np.asarray(q_U, np.float32) @ np.asarray(q_V, np.float32)) \
        * (SQ / math.sqrt(DH))
    Wk = (np.asarray(k_U, np.float32) @ np.asarray(k_V, np.float32)) * SK
    Wv = (np.asarray(v_U, np.float32) @ np.asarray(v_V, np.float32)) * SV
    oW = np.asarray(o_W, np.float32) * SO

    def hilo(a):
        hi = np.clip(a, -240, 240).astype(E4)
        lo = np.clip(a - hi.astype(np.float32), -240, 240).astype(E4)
        return hi, lo

    def w_image(WT):  # [2048, 256] -> [128, 16*256]
        return np.ascontiguousarray(
            WT.reshape(16, 128, DPC).transpose(1, 0, 2).reshape(128, 16 * DPC))

    def wo_image(oWcT):  # [256, 2048] -> [128, 2*2048]
        return np.ascontiguousarray(
            oWcT.reshape(HPC, 128, HIDDEN).transpose(1, 0, 2)
            .reshape(128, HPC * HIDDEN))

    xhi_img = x_image(x_hi)
    xlo_img = x_image(x_lo)
    in_maps = []
    for c in range(NCORES):
        sl = slice(c * DPC, (c + 1) * DPC)
        m = {"xhi": xhi_img, "xlo": xlo_img}
        for p, W in (("q", Wq), ("k", Wk), ("v", Wv)):
            hi, lo = hilo(np.ascontiguousarray(W[sl, :].T))
            m[f"w{p}hi"] = w_image(hi)
            if p == "v":
                m[f"w{p}lo"] = w_image(lo)
        hi, lo = hilo(np.ascontiguousarray(oW[:, sl].T))
        m["wohi"] = wo_image(hi)
        m["wolo"] = wo_image(lo)
        in_maps.append(m)
    return in_maps


def run(inputs, trace=False, tmpdir=None):
    from concourse.bass_utils import run_bass_kernel_spmd

    if "nc" not in _cache:
        _cache["nc"] = build_nc()
    nc = _cache["nc"]
    in_maps = host_prep(
        inputs["hidden_states"], inputs["q_V"], inputs["q_U"], inputs["k_V"],
        inputs["k_U"], inputs["v_V"], inputs["v_U"], inputs["o_W"],
    )
    res = run_bass_kernel_spmd(
        nc, in_maps, core_ids=list(range(NCORES)), trace=trace, tmpdir=tmpdir
    )
    acc = np.zeros((S, HIDDEN), np.float64)
    for c in range(NCORES):
        acc += res.results[c]["out"].astype(np.float64)
    out = (acc * OUT_DESCALE
           + np.asarray(inputs["o_b"], np.float64)[None, :]).astype(np.float32)
    return out.reshape(1, S, HIDDEN), res


def kernel(**inputs) -> np.ndarray:
    out, _ = run(inputs, trace=False)
    return out


# revision 4
# speedup vs baseline: 1.0320x; 1.0006x over previous
"""Low-rank self-attention TRN2 kernel v3 — fp8 DoubleRow everywhere.

Tensor-parallel over heads on 8 cores (heads 2c, 2c+1 on core c). Host merges
U@V into per-head effective QKV weights (same FLOPs as the low-rank form at
rank=hidden/2). All big matmuls run as fp8e4 DoubleRow (2x contraction per
pass):

  Stage AB (per 512-col chunk of x): q/k projections (dh-major, two-tier
    hi/lo fp8 on both x and W: hi*hi + lo*hi + hi*lo accumulated in one psum)
    and v projection (seq-major direct, same 3-tier scheme; no transposes).
    q/k evict as [64, 2, S] half-pair layout for the K=64 DoubleRow scores
    trick; v evicts as v_hi + v_lo residual pair (two-tier PV).
  Stage C attention per (qb of 1024 q-cols, head):
    scores S.T[k128, q1024] = DR(k_pair, q_pair)   (K=128 as 2x64 DoubleRow)
    P = exp(2^-15 * S.T) -> fp8e4                  (ACT, the critical engine)
    O += DR(v_hi pair, P pair) + DR(v_lo pair, P pair)   (psum f32)
    r += DR(ones, P pair)                          (row-sum broadcast to 128p)
    t = O/r (DVE divide); O_hi = e4(t); O_lo = e4(t - O_hi)
  o-proj (interleaved into the next qb's pair loop, shares the scores psum
    ring): out[t128, 2048] += DR(O_hi, wo_hi) + DR(O_lo, wo_hi)
    + DR(O_hi, wo_lo); evict f32->bf16 on gpsimd, DMA out.

Scales (powers of 2, folded so PE psums stay in range and exp arg is exact):
  Wq *= 512/sqrt(128), Wk *= 64, Wv *= 64, oW *= 32; exp scale 2^-15;
  host divides the summed partials by 2^11 (= 64*32) and adds o_b.
"""

import math
import sys

sys.path.insert(0, "/opt/trn_rl_repo")

import numpy as np
import ml_dtypes

HIDDEN = 2048
HEADS = 16
DH = 128
S = 4096
NCORES = 8
HPC = HEADS // NCORES   # 2 heads per core
DPC = HPC * DH          # 256 out dims per core per projection
QB = 1024               # q-block in attention
NCHUNK = S // 512       # 8 chunks of 512 seq cols
BF16 = ml_dtypes.bfloat16
E4 = ml_dtypes.float8_e4m3

SQ = 512.0
SK = 64.0
SV = 64.0
SO = 32.0
OUT_DESCALE = 1.0 / (SV * SO)   # applied on host
EXP_SCALE = 1.0 / (SQ * SK)     # 2^-15, exact

_cache = {}


def build_nc(debug=False):
    import concourse.bacc as bacc
    import concourse.mybir as mybir
    import concourse.tile as tile

    dt = mybir.dt
    AF = mybir.ActivationFunctionType
    ALU = mybir.AluOpType
    PM = mybir.MatmulPerfMode

    nc = bacc.Bacc(None, target_bir_lowering=False, debug=debug)
    xhi_d = nc.dram_tensor("xhi", [128, NCHUNK * 16 * 512], dt.float8e4,
                           kind="ExternalInput")
    xlo_d = nc.dram_tensor("xlo", [128, NCHUNK * 16 * 512], dt.float8e4,
                           kind="ExternalInput")
    # q/k are 2-tier ((x_hi+x_lo) @ W_hi); v is 3-tier (needs W_lo too)
    w_ds = {}
    for p, t_ in (("q", "hi"), ("k", "hi"), ("v", "hi"), ("v", "lo")):
        w_ds[p, t_] = nc.dram_tensor(f"w{p}{t_}", [128, 16 * 256],
                                     dt.float8e4, kind="ExternalInput")
    wo_ds = {t_: nc.dram_tensor(f"wo{t_}", [128, HPC * HIDDEN], dt.float8e4,
                                kind="ExternalInput") for t_ in ("hi", "lo")}
    out_d = nc.dram_tensor("out", [S, HIDDEN], dt.bfloat16, kind="ExternalOutput")

    with tile.TileContext(nc) as tc:
        with tc.tile_pool(name="persist", bufs=1) as pp, \
             tc.tile_pool(name="xp", bufs=3) as xp, \
             tc.tile_pool(name="xlp", bufs=3) as xlp, \
             tc.tile_pool(name="pt", bufs=34) as ptp, \
             tc.tile_pool(name="tsb", bufs=1) as tp, \
             tc.tile_pool(name="outst", bufs=3) as osp, \
             tc.tile_pool(name="big", bufs=2, space="PSUM") as bigp, \
             tc.tile_pool(name="oacc", bufs=1, space="PSUM") as oaccp, \
             tc.tile_pool(name="racc", bufs=1, space="PSUM") as raccp:

            # ---- persistent tiles (weight DMAs are emitted in the
            # prologue, interleaved with the first x chunks) ----
            w_s = {}
            for (p, t_), d in w_ds.items():
                w_s[p, t_] = pp.tile([128, 16, 256], dt.float8e4,
                                     tag=f"w{p}{t_}", name=f"w{p}{t_}s")
            wo_s = {}
            for t_, d in wo_ds.items():
                wo_s[t_] = pp.tile([128, HPC, HIDDEN], dt.float8e4,
                                   tag=f"wo{t_}", name=f"wo{t_}s")
            # q/k: [128, 2, S] with head h packed on partitions h*64..h*64+63
            qk_t = {}
            for p in ("q", "k"):
                qk_t[p] = pp.tile([128, 2, S], dt.float8e4,
                                  tag=f"{p}t", name=f"{p}t")
            v_t = {t_: pp.tile([128, S // 128, DPC], dt.float8e4, tag=f"v{t_}",
                               name=f"v{t_}t") for t_ in ("hi", "lo")}
            oT = {t_: pp.tile([128, HPC, S], dt.float8e4, tag=f"oT{t_}",
                              name=f"oT{t_}t") for t_ in ("hi", "lo")}
            ones_t = pp.tile([128, 2, 128], dt.float8e4, tag="ones", name="ones")
            nc.any.memset(ones_t[:], 1.0)
            # preload the exp table off the critical path
            dum_i = pp.tile([1, 16], dt.float32, tag="dumi", name="dumi")
            dum_o = pp.tile([1, 16], dt.bfloat16, tag="dumo", name="dumo")
            nc.any.memset(dum_i[:], 0.0)
            nc.scalar.activation(dum_o[:], dum_i[:], AF.Exp)

            # ---- Stage AB: QKV projection chunk emitters (all psums go
            # through the big ring so oacc/racc are attention-only) ----
            qk_tiers = (("hi", "hi"), ("lo", "hi"))
            v_tiers = (("hi", "hi"), ("lo", "hi"), ("hi", "lo"))

            def emit_chunk_dma(c, split=1):
                x_t = {}
                for t_, d in (("hi", xhi_d), ("lo", xlo_d)):
                    pool = xp if t_ == "hi" else xlp
                    x_t[t_] = pool.tile([128, 16, 512], dt.float8e4,
                                        tag=f"x{t_}", name=f"x{t_}_{c}")
                    sub = 8192 // split
                    for qtr in range(split):
                        nc.sync.dma_start(
                            out=x_t[t_][:, qtr * (16 // split):
                                        (qtr + 1) * (16 // split), :],
                            in_=d[:, c * 8192 + qtr * sub:
                                  c * 8192 + (qtr + 1) * sub],
                        )
                return x_t

            def emit_chunk_kq(c, x_t):
                for p in ("k", "q"):
                    if p == "k":
                        kpool, ktag = ((raccp, "racc") if c % 2 == 0
                                       else (oaccp, "oacc"))
                        ps = kpool.tile([128, 1024], dt.float32, tag=ktag,
                                        name=f"{p}ps_{c}")
                    else:
                        ps = bigp.tile([128, 1024], dt.float32, tag="big",
                                       name=f"{p}ps_{c}")
                    for b in range(HPC):
                        reg = ps[:, b * 512:(b + 1) * 512]
                        for ti, (xt_, wt_) in enumerate(qk_tiers):
                            for j2 in range(8):
                                nc.tensor.matmul(
                                    reg,
                                    w_s[p, wt_][:, 2 * j2:2 * j2 + 2,
                                                b * 128:(b + 1) * 128],
                                    x_t[xt_][:, 2 * j2:2 * j2 + 2, :],
                                    start=(ti == 0 and j2 == 0),
                                    stop=(ti == len(qk_tiers) - 1 and j2 == 7),
                                    perf_mode=PM.DoubleRow,
                                    skip_group_check=True,
                                )
                        # evict halves: psum rows 0:64 -> pair half 0,
                        # rows 64:128 -> pair half 1 (partitions b*64..)
                        bsl = slice(b * 64, (b + 1) * 64)
                        nc.vector.tensor_copy(
                            qk_t[p][bsl, 0, c * 512:(c + 1) * 512], reg[0:64, :])
                        nc.vector.tensor_copy(
                            qk_t[p][bsl, 1, c * 512:(c + 1) * 512], reg[64:128, :])
            def emit_chunk_v(c, x_t):
                # v psums use the oacc/racc rings, which are idle until the
                # first attention PVs (those start only after all chunks)
                vpool, vtag = ((oaccp, "oacc") if c % 2 == 0 else (raccp, "racc"))
                vps = vpool.tile([128, 1024], dt.float32, tag=vtag,
                                 name=f"vps_{c}")
                for s_ in range(4):
                    reg = vps[:, s_ * 256:(s_ + 1) * 256]
                    for ti, (xt_, wt_) in enumerate(v_tiers):
                        for j2 in range(8):
                            nc.tensor.matmul(
                                reg,
                                x_t[xt_][:, 2 * j2:2 * j2 + 2,
                                         s_ * 128:(s_ + 1) * 128],
                                w_s["v", wt_][:, 2 * j2:2 * j2 + 2, :],
                                start=(ti == 0 and j2 == 0),
                                stop=(ti == 2 and j2 == 7),
                                perf_mode=PM.DoubleRow,
                                skip_group_check=True,
                            )
                    kb = c * 4 + s_
                    nc.vector.tensor_copy(v_t["hi"][:, kb, :], reg)
                    nc.vector.tensor_tensor(v_t["lo"][:, kb, :], reg,
                                            v_t["hi"][:, kb, :], ALU.subtract)

            # ---- Stage C: attention + fused o-proj ----
            # o-proj work units for qb are emitted interleaved into qb+1's
            # pair loop (and flushed at the end for the last qb).
            pending_oproj = []
            staging = {}

            def emit_oproj_unit(qb, t_blk, half):
                tg = qb * (QB // 128) + t_blk
                if qb == S // QB - 1:
                    pool, ptag = [(bigp, "big"), (oaccp, "oacc"),
                                  (raccp, "racc")][(2 * t_blk + half) % 3]
                else:
                    pool, ptag = bigp, "big"
                ps = pool.tile([128, 1024], dt.float32, tag=ptag,
                               name=f"ops_{tg}_{half}")
                for nb2 in range(2):
                    nb = half * 2 + nb2
                    reg = ps[:, nb2 * 512:(nb2 + 1) * 512]
                    combos = (("hi", "hi"), ("lo", "hi"), ("hi", "lo"))
                    for ci, (ot_, wt_) in enumerate(combos):
                        nc.tensor.matmul(
                            reg,
                            oT[ot_][:, :, tg * 128:(tg + 1) * 128],
                            wo_s[wt_][:, :, nb * 512:(nb + 1) * 512],
                            start=(ci == 0),
                            stop=(ci == 2),
                            perf_mode=PM.DoubleRow,
                            skip_group_check=True,
                        )
                if half == 0:
                    st = osp.tile([128, 2048], dt.bfloat16, tag="outst",
                                  name=f"ost_{tg}")
                    staging[tg] = st
                else:
                    st = staging.pop(tg)
                if qb == S // QB - 1 and (2 * t_blk + half) % 2 == 0:
                    nc.scalar.copy(st[:, half * 1024:(half + 1) * 1024], ps[:])
                else:
                    nc.vector.tensor_copy(
                        st[:, half * 1024:(half + 1) * 1024], ps[:])
                if half == 1:
                    nc.sync.dma_start(out=out_d[tg * 128:(tg + 1) * 128, :],
                                      in_=st[:])

            def drain_oproj():
                if pending_oproj:
                    qb, t_blk, half = pending_oproj.pop(0)
                    emit_oproj_unit(qb, t_blk, half)

            # Attention is emitted as sweeps (one (qb, h) = 16 pair units).
            # PV/r of sweep s-1 interleave with scores/exp of sweep s (a
            # full-sweep software pipeline), so the PE's in-order queue never
            # stalls the ACT feed; sweep 0's scores chase the QKV chunk
            # production, overlapping stage AB with attention.
            acc_ps = {}

            def emit_scores(u):
                qb, h, pr = u
                p_t = ptp.tile([128, 2, QB], dt.float8e4, tag="pt",
                               name=f"pt_{qb}_{h}_{pr}")
                for i in range(2):
                    kb = 2 * pr + i
                    sc = bigp.tile([128, QB], dt.float32, tag="big",
                                   name=f"sc_{qb}_{h}_{kb}")
                    hsl = slice(h * 64, (h + 1) * 64)
                    for j in range(2):
                        nc.tensor.matmul(
                            sc[:, j * 512:(j + 1) * 512],
                            qk_t["k"][hsl, :, kb * 128:(kb + 1) * 128],
                            qk_t["q"][hsl, :, qb * QB + j * 512:
                                      qb * QB + (j + 1) * 512],
                            start=True,
                            stop=True,
                            perf_mode=PM.DoubleRow,
                            skip_group_check=True,
                        )
                    nc.scalar.activation(p_t[:, i, :], sc[:], AF.Exp,
                                         scale=EXP_SCALE)
                return p_t

            def emit_pv(u, p_t):
                qb, h, pr = u
                if pr == 0:
                    acc_ps[qb, h] = (
                        oaccp.tile([128, QB], dt.float32, tag="oacc",
                                   name=f"ops_{qb}_{h}"),
                        raccp.tile([128, QB], dt.float32, tag="racc",
                                   name=f"rps_{qb}_{h}"),
                    )
                o_ps, r_ps = acc_ps[qb, h]
                for j in range(2):
                    jsl = slice(j * 512, (j + 1) * 512)
                    for vi, vt_ in enumerate(("hi", "lo")):
                        nc.tensor.matmul(
                            o_ps[:, jsl],
                            v_t[vt_][:, 2 * pr:2 * pr + 2,
                                     h * 128:(h + 1) * 128],
                            p_t[:, :, jsl],
                            start=(pr == 0 and vi == 0),
                            stop=(pr == 15 and vi == 1),
                            perf_mode=PM.DoubleRow,
                            skip_group_check=True,
                        )
                    nc.tensor.matmul(
                        r_ps[:, jsl],
                        ones_t[:],
                        p_t[:, :, jsl],
                        start=(pr == 0),
                        stop=(pr == 15),
                        perf_mode=PM.DoubleRow,
                        skip_group_check=True,
                    )
                if pr == 15:
                    # softmax normalize + two-tier O eviction
                    del acc_ps[qb, h]
                    rb_sb = tp.tile([128, QB], dt.float32, tag="rbin",
                                    name=f"rb_{qb}_{h}")
                    nc.vector.reciprocal(rb_sb[:], r_ps[:])
                    t_sb = tp.tile([128, QB], dt.bfloat16, tag="tsb",
                                   name=f"tsb_{qb}_{h}")
                    nc.vector.tensor_tensor(t_sb[:], o_ps[:], rb_sb[:],
                                            ALU.mult)
                    ohi = oT["hi"][:, h, qb * QB:(qb + 1) * QB]
                    nc.gpsimd.tensor_copy(ohi, t_sb[:])
                    nc.gpsimd.tensor_tensor(
                        oT["lo"][:, h, qb * QB:(qb + 1) * QB],
                        t_sb[:], ohi, ALU.subtract)
                    if h == HPC - 1:
                        for t_blk in range(QB // 128):
                            pending_oproj.append((qb, t_blk, 0))
                            pending_oproj.append((qb, t_blk, 1))

            sweeps = [(qb, h) for qb in range(S // QB) for h in range(HPC)]
            sc_q = []

            def emit_sc_unit(sw, pr):
                qb, h = sw
                sc_q.append(((qb, h, pr), emit_scores((qb, h, pr))))

            def emit_pv_unit():
                u, p_t = sc_q.pop(0)
                emit_pv(u, p_t)

            # Prologue: stage AB chunks with sweeps 0 AND 1 prefetching their
            # scores/exp behind the chunk production (both depend only on the
            # k/v/q chunks produced so far; no PVs run during AB so the
            # oacc/racc rings are free for the v psums).
            x0 = emit_chunk_dma(0)
            for p, t_ in (("k", "hi"), ("q", "hi"), ("v", "hi"), ("v", "lo")):
                nc.scalar.dma_start(out=w_s[p, t_][:], in_=w_ds[p, t_][:])
            x1 = emit_chunk_dma(1)
            for t_ in ("hi", "lo"):
                nc.scalar.dma_start(out=wo_s[t_][:], in_=wo_ds[t_][:])
            x_tiles = {0: x0, 1: x1}
            emit_chunk_kq(0, x0)
            emit_chunk_kq(1, x1)
            emit_sc_unit(sweeps[0], 0)
            emit_sc_unit(sweeps[0], 1)
            for c in range(2, NCHUNK):
                x_tiles[c] = emit_chunk_dma(c)
                emit_chunk_v(c - 2, x_tiles.pop(c - 2))
                emit_chunk_kq(c, x_tiles[c])
                emit_sc_unit(sweeps[0], 2 * (c - 1))
                emit_sc_unit(sweeps[0], 2 * (c - 1) + 1)
                if c >= 2:
                    emit_sc_unit(sweeps[1], 2 * (c - 2))
                    emit_sc_unit(sweeps[1], 2 * (c - 2) + 1)
                if c >= 4:
                    emit_sc_unit(sweeps[2], 2 * (c - 4))
                    emit_sc_unit(sweeps[2], 2 * (c - 4) + 1)
            emit_chunk_v(NCHUNK - 2, x_tiles.pop(NCHUNK - 2))
            emit_chunk_v(NCHUNK - 1, x_tiles.pop(NCHUNK - 1))
            # post-AB: finish the prefetched sweeps' scores while draining
            # queued PVs (two per unit) to bound the pipeline depth
            tail_units = ([(sweeps[0], pr) for pr in (14, 15)]
                          + [(sweeps[1], pr)
                             for pr in range(2 * (NCHUNK - 2), 16)]
                          + [(sweeps[2], pr)
                             for pr in range(2 * (NCHUNK - 4), 16)])
            for sw, pr in tail_units:
                emit_pv_unit()
                emit_sc_unit(sw, pr)
                if len(sc_q) > 24:
                    emit_pv_unit()
            # Steady state: emit scores of sweep si while draining queued PVs
            # (at most 2 per step) until the pipeline lag settles at one sweep.
            for si in range(3, len(sweeps)):
                tgt = 17 if si < len(sweeps) - 2 else (10 if si == 6 else 4)
                for pr in range(16):
                    if pr == 0:
                        emit_sc_unit(sweeps[si], 0)
                        emit_sc_unit(sweeps[si], 1)
                    elif pr < 15:
                        emit_sc_unit(sweeps[si], pr + 1)
                    emit_pv_unit()
                    if len(sc_q) > tgt:
                        emit_pv_unit()
                    if pr % 2 == 1:
                        drain_oproj()
            while sc_q:
                emit_pv_unit()
                drain_oproj()
            while pending_oproj:
                drain_oproj()
    nc.finalize()
    return nc


def host_prep(hidden_states, q_V, q_U, k_V, k_U, v_V, v_U, o_W):
    """Per-core input maps: fp8 hi/lo splits + swizzled layouts."""
    x = np.asarray(hidden_states, np.float32).reshape(S, HIDDEN)
    xT = np.ascontiguousarray(x.T)                      # [HIDDEN, S]
    x_hi = np.clip(xT, -240, 240).astype(E4)
    x_lo = np.clip(xT - x_hi.astype(np.float32), -240, 240).astype(E4)

    def x_image(a):  # [2048, 4096] -> [128, NCHUNK*16*512] chunk-major swizzle
        # xs[p, c, blk, col] = a[blk*128+p, c*512+col]
        b = a.reshape(16, 128, NCHUNK, 512)             # [blk, p, c, col]
        return np.ascontiguousarray(
            b.transpose(1, 2, 0, 3).reshape(128, NCHUNK * 16 * 512))

    Wq = (np.asarray(q_U, np.float32) @ np.asarray(q_V, np.float32)) \
        * (SQ / math.sqrt(DH))
    Wk = (np.asarray(k_U, np.float32) @ np.asarray(k_V, np.float32)) * SK
    Wv = (np.asarray(v_U, np.float32) @ np.asarray(v_V, np.float32)) * SV
    oW = np.asarray(o_W, np.float32) * SO

    def hilo(a):
        hi = np.clip(a, -240, 240).astype(E4)
        lo = np.clip(a - hi.astype(np.float32), -240, 240).astype(E4)
        return hi, lo

    def w_image(WT):  # [2048, 256] -> [128, 16*256]
        return np.ascontiguousarray(
            WT.reshape(16, 128, DPC).transpose(1, 0, 2).reshape(128, 16 * DPC))

    def wo_image(oWcT):  # [256, 2048] -> [128, 2*2048]
        return np.ascontiguousarray(
            oWcT.reshape(HPC, 128, HIDDEN).transpose(1, 0, 2)
            .reshape(128, HPC * HIDDEN))

    xhi_img = x_image(x_hi)
    xlo_img = x_image(x_lo)
    in_maps = []
    for c in range(NCORES):
        sl = slice(c * DPC, (c + 1) * DPC)
        m = {"xhi": xhi_img, "xlo": xlo_img}
        for p, W in (("q", Wq), ("k", Wk), ("v", Wv)):
            hi, lo = hilo(np.ascontiguousarray(W[sl, :].T))
            m[f"w{p}hi"] = w_image(hi)
            if p == "v":
                m[f"w{p}lo"] = w_image(lo)
        hi, lo = hilo(np.ascontiguousarray(oW[:, sl].T))
        m["wohi"] = wo_image(hi)
        m["wolo"] = wo_image(lo)
        in_maps.append(m)
    return in_maps


def run(inputs, trace=False, tmpdir=None):
    from concourse.bass_utils import run_bass_kernel_spmd

    if "nc" not in _cache:
        _cache["nc"] = build_nc()
    nc = _cache["nc"]
    in_maps = host_prep(
        inputs["hidden_states"], inputs["q_V"], inputs["q_U"], inputs["k_V"],
        inputs["k_U"], inputs["v_V"], inputs["v_U"], inputs["o_W"],
    )
    res = run_bass_kernel_spmd(
        nc, in_maps, core_ids=list(range(NCORES)), trace=trace, tmpdir=tmpdir
    )
    acc = np.zeros((S, HIDDEN), np.float64)
    for c in range(NCORES):
        acc += res.results[c]["out"].astype(np.float64)
    out = (acc * OUT_DESCALE
           + np.asarray(inputs["o_b"], np.float64)[None, :]).astype(np.float32)
    return out.reshape(1, S, HIDDEN), res


def kernel(**inputs) -> np.ndarray:
    out, _ = run(inputs, trace=False)
    return out


# revision 5
# speedup vs baseline: 1.0358x; 1.0037x over previous
"""Low-rank self-attention TRN2 kernel v3 — fp8 DoubleRow everywhere.

Tensor-parallel over heads on 8 cores (heads 2c, 2c+1 on core c). Host merges
U@V into per-head effective QKV weights (same FLOPs as the low-rank form at
rank=hidden/2). All big matmuls run as fp8e4 DoubleRow (2x contraction per
pass):

  Stage AB (per 512-col chunk of x): q/k projections (dh-major, two-tier
    hi/lo fp8 on both x and W: hi*hi + lo*hi + hi*lo accumulated in one psum)
    and v projection (seq-major direct, same 3-tier scheme; no transposes).
    q/k evict as [64, 2, S] half-pair layout for the K=64 DoubleRow scores
    trick; v evicts as v_hi + v_lo residual pair (two-tier PV).
  Stage C attention per (qb of 1024 q-cols, head):
    scores S.T[k128, q1024] = DR(k_pair, q_pair)   (K=128 as 2x64 DoubleRow)
    P = exp(2^-15 * S.T) -> fp8e4                  (ACT, the critical engine)
    O += DR(v_hi pair, P pair) + DR(v_lo pair, P pair)   (psum f32)
    r += DR(ones, P pair)                          (row-sum broadcast to 128p)
    t = O/r (DVE divide); O_hi = e4(t); O_lo = e4(t - O_hi)
  o-proj (interleaved into the next qb's pair loop, shares the scores psum
    ring): out[t128, 2048] += DR(O_hi, wo_hi) + DR(O_lo, wo_hi)
    + DR(O_hi, wo_lo); evict f32->bf16 on gpsimd, DMA out.

Scales (powers of 2, folded so PE psums stay in range and exp arg is exact):
  Wq *= 512/sqrt(128), Wk *= 64, Wv *= 64, oW *= 32; exp scale 2^-15;
  host divides the summed partials by 2^11 (= 64*32) and adds o_b.
"""

import math
import sys

sys.path.insert(0, "/opt/trn_rl_repo")

import numpy as np
import ml_dtypes

HIDDEN = 2048
HEADS = 16
DH = 128
S = 4096
NCORES = 8
HPC = HEADS // NCORES   # 2 heads per core
DPC = HPC * DH          # 256 out dims per core per projection
QB = 1024               # q-block in attention
NCHUNK = S // 512       # 8 chunks of 512 seq cols
BF16 = ml_dtypes.bfloat16
E4 = ml_dtypes.float8_e4m3

SQ = 512.0
SK = 64.0
SV = 64.0
SO = 32.0
OUT_DESCALE = 1.0 / (SV * SO)   # applied on host
EXP_SCALE = 1.0 / (SQ * SK)     # 2^-15, exact

_cache = {}


def build_nc(debug=False):
    import concourse.bacc as bacc
    import concourse.mybir as mybir
    import concourse.tile as tile

    dt = mybir.dt
    AF = mybir.ActivationFunctionType
    ALU = mybir.AluOpType
    PM = mybir.MatmulPerfMode

    nc = bacc.Bacc(None, target_bir_lowering=False, debug=debug)
    xhi_d = nc.dram_tensor("xhi", [128, NCHUNK * 16 * 512], dt.float8e4,
                           kind="ExternalInput")
    xlo_d = nc.dram_tensor("xlo", [128, NCHUNK * 16 * 512], dt.float8e4,
                           kind="ExternalInput")
    # q/k are 2-tier ((x_hi+x_lo) @ W_hi); v is 3-tier (needs W_lo too)
    w_ds = {}
    for p, t_ in (("q", "hi"), ("k", "hi"), ("v", "hi"), ("v", "lo")):
        w_ds[p, t_] = nc.dram_tensor(f"w{p}{t_}", [128, 16 * 256],
                                     dt.float8e4, kind="ExternalInput")
    wo_ds = {t_: nc.dram_tensor(f"wo{t_}", [128, HPC * HIDDEN], dt.float8e4,
                                kind="ExternalInput") for t_ in ("hi", "lo")}
    out_d = nc.dram_tensor("out", [S, HIDDEN], dt.bfloat16, kind="ExternalOutput")

    with tile.TileContext(nc) as tc:
        with tc.tile_pool(name="persist", bufs=1) as pp, \
             tc.tile_pool(name="xp", bufs=3) as xp, \
             tc.tile_pool(name="xlp", bufs=3) as xlp, \
             tc.tile_pool(name="pt", bufs=34) as ptp, \
             tc.tile_pool(name="tsb", bufs=1) as tp, \
             tc.tile_pool(name="outst", bufs=3) as osp, \
             tc.tile_pool(name="big", bufs=2, space="PSUM") as bigp, \
             tc.tile_pool(name="oacc", bufs=1, space="PSUM") as oaccp, \
             tc.tile_pool(name="racc", bufs=1, space="PSUM") as raccp:

            # ---- persistent tiles (weight DMAs are emitted in the
            # prologue, interleaved with the first x chunks) ----
            w_s = {}
            for (p, t_), d in w_ds.items():
                w_s[p, t_] = pp.tile([128, 16, 256], dt.float8e4,
                                     tag=f"w{p}{t_}", name=f"w{p}{t_}s")
            wo_s = {}
            for t_, d in wo_ds.items():
                wo_s[t_] = pp.tile([128, HPC, HIDDEN], dt.float8e4,
                                   tag=f"wo{t_}", name=f"wo{t_}s")
            # q/k: [128, 2, S] with head h packed on partitions h*64..h*64+63
            qk_t = {}
            for p in ("q", "k"):
                qk_t[p] = pp.tile([128, 2, S], dt.float8e4,
                                  tag=f"{p}t", name=f"{p}t")
            v_t = {t_: pp.tile([128, S // 128, DPC], dt.float8e4, tag=f"v{t_}",
                               name=f"v{t_}t") for t_ in ("hi", "lo")}
            oT = {t_: pp.tile([128, HPC, S], dt.float8e4, tag=f"oT{t_}",
                              name=f"oT{t_}t") for t_ in ("hi", "lo")}
            ones_t = pp.tile([128, 2, 128], dt.float8e4, tag="ones", name="ones")
            nc.any.memset(ones_t[:], 1.0)
            # preload the exp table off the critical path
            dum_i = pp.tile([1, 16], dt.float32, tag="dumi", name="dumi")
            dum_o = pp.tile([1, 16], dt.bfloat16, tag="dumo", name="dumo")
            nc.any.memset(dum_i[:], 0.0)
            nc.scalar.activation(dum_o[:], dum_i[:], AF.Exp)

            # ---- Stage AB: QKV projection chunk emitters (all psums go
            # through the big ring so oacc/racc are attention-only) ----
            qk_tiers = (("hi", "hi"), ("lo", "hi"))
            v_tiers = (("hi", "hi"), ("lo", "hi"), ("hi", "lo"))

            def emit_chunk_dma(c, split=1):
                x_t = {}
                for t_, d in (("hi", xhi_d), ("lo", xlo_d)):
                    pool = xp if t_ == "hi" else xlp
                    x_t[t_] = pool.tile([128, 16, 512], dt.float8e4,
                                        tag=f"x{t_}", name=f"x{t_}_{c}")
                    sub = 8192 // split
                    for qtr in range(split):
                        nc.sync.dma_start(
                            out=x_t[t_][:, qtr * (16 // split):
                                        (qtr + 1) * (16 // split), :],
                            in_=d[:, c * 8192 + qtr * sub:
                                  c * 8192 + (qtr + 1) * sub],
                        )
                return x_t

            def emit_chunk_kq(c, x_t):
                for p in ("k", "q"):
                    if p == "k":
                        kpool, ktag = ((raccp, "racc") if c % 2 == 0
                                       else (oaccp, "oacc"))
                        ps = kpool.tile([128, 1024], dt.float32, tag=ktag,
                                        name=f"{p}ps_{c}")
                    else:
                        ps = bigp.tile([128, 1024], dt.float32, tag="big",
                                       name=f"{p}ps_{c}")
                    for b in range(HPC):
                        reg = ps[:, b * 512:(b + 1) * 512]
                        for ti, (xt_, wt_) in enumerate(qk_tiers):
                            for j2 in range(8):
                                nc.tensor.matmul(
                                    reg,
                                    w_s[p, wt_][:, 2 * j2:2 * j2 + 2,
                                                b * 128:(b + 1) * 128],
                                    x_t[xt_][:, 2 * j2:2 * j2 + 2, :],
                                    start=(ti == 0 and j2 == 0),
                                    stop=(ti == len(qk_tiers) - 1 and j2 == 7),
                                    perf_mode=PM.DoubleRow,
                                    skip_group_check=True,
                                )
                        # evict halves: psum rows 0:64 -> pair half 0,
                        # rows 64:128 -> pair half 1 (partitions b*64..)
                        bsl = slice(b * 64, (b + 1) * 64)
                        nc.vector.tensor_copy(
                            qk_t[p][bsl, 0, c * 512:(c + 1) * 512], reg[0:64, :])
                        nc.vector.tensor_copy(
                            qk_t[p][bsl, 1, c * 512:(c + 1) * 512], reg[64:128, :])
            def emit_chunk_v(c, x_t):
                # v psums use the oacc/racc rings, which are idle until the
                # first attention PVs (those start only after all chunks)
                vpool, vtag = ((oaccp, "oacc") if c % 2 == 0 else (raccp, "racc"))
                vps = vpool.tile([128, 1024], dt.float32, tag=vtag,
                                 name=f"vps_{c}")
                for s_ in range(4):
                    reg = vps[:, s_ * 256:(s_ + 1) * 256]
                    for ti, (xt_, wt_) in enumerate(v_tiers):
                        for j2 in range(8):
                            nc.tensor.matmul(
                                reg,
                                x_t[xt_][:, 2 * j2:2 * j2 + 2,
                                         s_ * 128:(s_ + 1) * 128],
                                w_s["v", wt_][:, 2 * j2:2 * j2 + 2, :],
                                start=(ti == 0 and j2 == 0),
                                stop=(ti == 2 and j2 == 7),
                                perf_mode=PM.DoubleRow,
                                skip_group_check=True,
                            )
                    kb = c * 4 + s_
                    nc.vector.tensor_copy(v_t["hi"][:, kb, :], reg)
                    nc.vector.tensor_tensor(v_t["lo"][:, kb, :], reg,
                                            v_t["hi"][:, kb, :], ALU.subtract)

            # ---- Stage C: attention + fused o-proj ----
            # o-proj work units for qb are emitted interleaved into qb+1's
            # pair loop (and flushed at the end for the last qb).
            pending_oproj = []
            staging = {}

            def emit_oproj_unit(qb, t_blk, half):
                tg = qb * (QB // 128) + t_blk
                if qb == S // QB - 1:
                    pool, ptag = [(bigp, "big"), (oaccp, "oacc"),
                                  (raccp, "racc")][(2 * t_blk + half) % 3]
                else:
                    pool, ptag = bigp, "big"
                ps = pool.tile([128, 1024], dt.float32, tag=ptag,
                               name=f"ops_{tg}_{half}")
                for nb2 in range(2):
                    nb = half * 2 + nb2
                    reg = ps[:, nb2 * 512:(nb2 + 1) * 512]
                    combos = (("hi", "hi"), ("lo", "hi"), ("hi", "lo"))
                    for ci, (ot_, wt_) in enumerate(combos):
                        nc.tensor.matmul(
                            reg,
                            oT[ot_][:, :, tg * 128:(tg + 1) * 128],
                            wo_s[wt_][:, :, nb * 512:(nb + 1) * 512],
                            start=(ci == 0),
                            stop=(ci == 2),
                            perf_mode=PM.DoubleRow,
                            skip_group_check=True,
                        )
                if half == 0:
                    st = osp.tile([128, 2048], dt.bfloat16, tag="outst",
                                  name=f"ost_{tg}")
                    staging[tg] = st
                else:
                    st = staging.pop(tg)
                if qb == S // QB - 1 and (2 * t_blk + half) % 2 == 0:
                    nc.scalar.copy(st[:, half * 1024:(half + 1) * 1024], ps[:])
                else:
                    nc.vector.tensor_copy(
                        st[:, half * 1024:(half + 1) * 1024], ps[:])
                if half == 1:
                    nc.sync.dma_start(out=out_d[tg * 128:(tg + 1) * 128, :],
                                      in_=st[:])

            def drain_oproj():
                if pending_oproj:
                    qb, t_blk, half = pending_oproj.pop(0)
                    emit_oproj_unit(qb, t_blk, half)

            # Attention is emitted as sweeps (one (qb, h) = 16 pair units).
            # PV/r of sweep s-1 interleave with scores/exp of sweep s (a
            # full-sweep software pipeline), so the PE's in-order queue never
            # stalls the ACT feed; sweep 0's scores chase the QKV chunk
            # production, overlapping stage AB with attention.
            acc_ps = {}

            def emit_scores(u):
                qb, h, pr = u
                p_t = ptp.tile([128, 2, QB], dt.float8e4, tag="pt",
                               name=f"pt_{qb}_{h}_{pr}")
                for i in range(2):
                    kb = 2 * pr + i
                    sc = bigp.tile([128, QB], dt.float32, tag="big",
                                   name=f"sc_{qb}_{h}_{kb}")
                    hsl = slice(h * 64, (h + 1) * 64)
                    for j in range(2):
                        nc.tensor.matmul(
                            sc[:, j * 512:(j + 1) * 512],
                            qk_t["k"][hsl, :, kb * 128:(kb + 1) * 128],
                            qk_t["q"][hsl, :, qb * QB + j * 512:
                                      qb * QB + (j + 1) * 512],
                            start=True,
                            stop=True,
                            perf_mode=PM.DoubleRow,
                            skip_group_check=True,
                        )
                    nc.scalar.activation(p_t[:, i, :], sc[:], AF.Exp,
                                         scale=EXP_SCALE)
                return p_t

            def emit_pv(u, p_t):
                qb, h, pr = u
                if pr == 0:
                    acc_ps[qb, h] = (
                        oaccp.tile([128, QB], dt.float32, tag="oacc",
                                   name=f"ops_{qb}_{h}"),
                        raccp.tile([128, QB], dt.float32, tag="racc",
                                   name=f"rps_{qb}_{h}"),
                    )
                o_ps, r_ps = acc_ps[qb, h]
                for j in range(2):
                    jsl = slice(j * 512, (j + 1) * 512)
                    for vi, vt_ in enumerate(("hi", "lo")):
                        nc.tensor.matmul(
                            o_ps[:, jsl],
                            v_t[vt_][:, 2 * pr:2 * pr + 2,
                                     h * 128:(h + 1) * 128],
                            p_t[:, :, jsl],
                            start=(pr == 0 and vi == 0),
                            stop=(pr == 15 and vi == 1),
                            perf_mode=PM.DoubleRow,
                            skip_group_check=True,
                        )
                    nc.tensor.matmul(
                        r_ps[:, jsl],
                        ones_t[:],
                        p_t[:, :, jsl],
                        start=(pr == 0),
                        stop=(pr == 15),
                        perf_mode=PM.DoubleRow,
                        skip_group_check=True,
                    )
                if pr == 15:
                    # softmax normalize + two-tier O eviction
                    del acc_ps[qb, h]
                    rb_sb = tp.tile([128, QB], dt.float32, tag="rbin",
                                    name=f"rb_{qb}_{h}")
                    nc.vector.reciprocal(rb_sb[:], r_ps[:])
                    t_sb = tp.tile([128, QB], dt.bfloat16, tag="tsb",
                                   name=f"tsb_{qb}_{h}")
                    nc.vector.tensor_tensor(t_sb[:], o_ps[:], rb_sb[:],
                                            ALU.mult)
                    ohi = oT["hi"][:, h, qb * QB:(qb + 1) * QB]
                    nc.gpsimd.tensor_copy(ohi, t_sb[:])
                    nc.gpsimd.tensor_tensor(
                        oT["lo"][:, h, qb * QB:(qb + 1) * QB],
                        t_sb[:], ohi, ALU.subtract)
                    if h == HPC - 1:
                        for t_blk in range(QB // 128):
                            pending_oproj.append((qb, t_blk, 0))
                            pending_oproj.append((qb, t_blk, 1))

            sweeps = [(qb, h) for qb in range(S // QB) for h in range(HPC)]
            sc_q = []

            def emit_sc_unit(sw, pr):
                qb, h = sw
                sc_q.append(((qb, h, pr), emit_scores((qb, h, pr))))

            def emit_pv_unit():
                u, p_t = sc_q.pop(0)
                emit_pv(u, p_t)

            # Prologue: stage AB chunks with sweeps 0 AND 1 prefetching their
            # scores/exp behind the chunk production (both depend only on the
            # k/v/q chunks produced so far; no PVs run during AB so the
            # oacc/racc rings are free for the v psums).
            x0 = emit_chunk_dma(0)
            for p, t_ in (("k", "hi"), ("q", "hi"), ("v", "hi"), ("v", "lo")):
                nc.scalar.dma_start(out=w_s[p, t_][:], in_=w_ds[p, t_][:])
            x1 = emit_chunk_dma(1)
            for t_ in ("hi", "lo"):
                nc.scalar.dma_start(out=wo_s[t_][:], in_=wo_ds[t_][:])
            x_tiles = {0: x0, 1: x1}
            emit_chunk_kq(0, x0)
            emit_chunk_kq(1, x1)
            emit_sc_unit(sweeps[0], 0)
            emit_sc_unit(sweeps[0], 1)
            emit_sc_unit(sweeps[0], 2)
            emit_sc_unit(sweeps[0], 3)
            emit_sc_unit(sweeps[1], 0)
            emit_sc_unit(sweeps[1], 1)
            for c in range(2, NCHUNK):
                x_tiles[c] = emit_chunk_dma(c)
                emit_chunk_v(c - 2, x_tiles.pop(c - 2))
                emit_chunk_kq(c, x_tiles[c])
                # emit right at the dependency frontier: pair 2c needs only
                # k-chunk c (just produced) and q-chunks 0-1
                emit_sc_unit(sweeps[0], 2 * c)
                emit_sc_unit(sweeps[0], 2 * c + 1)
                emit_sc_unit(sweeps[1], 2 * (c - 1))
                emit_sc_unit(sweeps[1], 2 * (c - 1) + 1)
                if c >= 6:
                    emit_sc_unit(sweeps[2], 2 * (c - 6))
                    emit_sc_unit(sweeps[2], 2 * (c - 6) + 1)
            emit_chunk_v(NCHUNK - 2, x_tiles.pop(NCHUNK - 2))
            emit_chunk_v(NCHUNK - 1, x_tiles.pop(NCHUNK - 1))
            # post-AB: finish the prefetched sweeps' scores while draining
            # queued PVs (two per unit) to bound the pipeline depth
            tail_units = ([(sweeps[1], pr) for pr in (14, 15)]
                          + [(sweeps[2], pr)
                             for pr in range(2 * (NCHUNK - 6), 16)])
            for sw, pr in tail_units:
                emit_pv_unit()
                emit_sc_unit(sw, pr)
                if len(sc_q) > 24:
                    emit_pv_unit()
            # Steady state: emit scores of sweep si while draining queued PVs
            # (at most 2 per step) until the pipeline lag settles at one sweep.
            for si in range(3, len(sweeps)):
                tgt = 17 if si < len(sweeps) - 2 else (10 if si == 6 else 4)
                for pr in range(16):
                    if pr == 0:
                        emit_sc_unit(sweeps[si], 0)
                        emit_sc_unit(sweeps[si], 1)
                    elif pr < 15:
                        emit_sc_unit(sweeps[si], pr + 1)
                    emit_pv_unit()
                    if len(sc_q) > tgt:
                        emit_pv_unit()
                    if pr % 2 == 1:
                        drain_oproj()
            while sc_q:
                emit_pv_unit()
                drain_oproj()
            while pending_oproj:
                drain_oproj()
    nc.finalize()
    return nc


def host_prep(hidden_states, q_V, q_U, k_V, k_U, v_V, v_U, o_W):
    """Per-core input maps: fp8 hi/lo splits + swizzled layouts."""
    x = np.asarray(hidden_states, np.float32).reshape(S, HIDDEN)
    xT = np.ascontiguousarray(x.T)                      # [HIDDEN, S]
    x_hi = np.clip(xT, -240, 240).astype(E4)
    x_lo = np.clip(xT - x_hi.astype(np.float32), -240, 240).astype(E4)

    def x_image(a):  # [2048, 4096] -> [128, NCHUNK*16*512] chunk-major swizzle
        # xs[p, c, blk, col] = a[blk*128+p, c*512+col]
        b = a.reshape(16, 128, NCHUNK, 512)             # [blk, p, c, col]
        return np.ascontiguousarray(
            b.transpose(1, 2, 0, 3).reshape(128, NCHUNK * 16 * 512))

    Wq = (np.asarray(q_U, np.float32) @ np.asarray(q_V, np.float32)) \
        * (SQ / math.sqrt(DH))
    Wk = (np.asarray(k_U, np.float32) @ np.asarray(k_V, np.float32)) * SK
    Wv = (np.asarray(v_U, np.float32) @ np.asarray(v_V, np.float32)) * SV
    oW = np.asarray(o_W, np.float32) * SO

    def hilo(a):
        hi = np.clip(a, -240, 240).astype(E4)
        lo = np.clip(a - hi.astype(np.float32), -240, 240).astype(E4)
        return hi, lo

    def w_image(WT):  # [2048, 256] -> [128, 16*256]
        return np.ascontiguousarray(
            WT.reshape(16, 128, DPC).transpose(1, 0, 2).reshape(128, 16 * DPC))

    def wo_image(oWcT):  # [256, 2048] -> [128, 2*2048]
        return np.ascontiguousarray(
            oWcT.reshape(HPC, 128, HIDDEN).transpose(1, 0, 2)
            .reshape(128, HPC * HIDDEN))

    xhi_img = x_image(x_hi)
    xlo_img = x_image(x_lo)
    in_maps = []
    for c in range(NCORES):
        sl = slice(c * DPC, (c + 1) * DPC)
        m = {"xhi": xhi_img, "xlo": xlo_img}
        for p, W in (("q", Wq), ("k", Wk), ("v", Wv)):
            hi, lo = hilo(np.ascontiguousarray(W[sl, :].T))
            m[f"w{p}hi"] = w_image(hi)
            if p == "v":
                m[f"w{p}lo"] = w_image(lo)
        hi, lo = hilo(np.ascontiguousarray(oW[:, sl].T))
        m["wohi"] = wo_image(hi)
        m["wolo"] = wo_image(lo)
        in_maps.append(m)
    return in_maps


def run(inputs, trace=False, tmpdir=None):
    from concourse.bass_utils import run_bass_kernel_spmd

    if "nc" not in _cache:
        _cache["nc"] = build_nc()
    nc = _cache["nc"]
    in_maps = host_prep(
        inputs["hidden_states"], inputs["q_V"], inputs["q_U"], inputs["k_V"],
        inputs["k_U"], inputs["v_V"], inputs["v_U"], inputs["o_W"],
    )
    res = run_bass_kernel_spmd(
        nc, in_maps, core_ids=list(range(NCORES)), trace=trace, tmpdir=tmpdir
    )
    acc = np.zeros((S, HIDDEN), np.float64)
    for c in range(NCORES):
        acc += res.results[c]["out"].astype(np.float64)
    out = (acc * OUT_DESCALE
           + np.asarray(inputs["o_b"], np.float64)[None, :]).astype(np.float32)
    return out.reshape(1, S, HIDDEN), res


def kernel(**inputs) -> np.ndarray:
    out, _ = run(inputs, trace=False)
    return out
